# revision 2
# baseline (speedup 1.0000x reference)
"""Trainium2 Bass kernel for T5-style cross-attention, sharded over 8 NeuronCores.

Sharding: tensor-parallel over heads (16 heads -> 2 per core). Each core
computes Q/K/V projections for its 2 heads (full batch), flash-style
attention with multiplicative exp(position_bias), and a partial output
projection against its row-slice of Wo. The host sums the 8 fp16 partial
outputs (the unshard step for a row-sharded Wo).

Latency-oriented design (v5) (ablations showed the old kernel was bound
by the scores->exp->mul->ctx dependency chain serializing the in-order PE
queue, plus DMA descriptor-generation serializing on one engine queue):
- Heads-packed score tiles: one [128, 2*QW] PSUM tile per (qw,bi,ktile)
  holds both heads; one exp and one exp(bias)-multiply per group.
- Software pipelining: ctx matmuls for group g issue after scores of
  group g+LAG, so the PE never stalls on the exp/mul chain (psbig bufs=3
  gives exp a 3-group window; sattn holds LAG groups of attn tiles).
- Projections for batch bi are emitted inside the first q-window's batch
  loop, so exp/DVE work starts after only 1/4 of the projections.
- DMA spread across engine queues (x->sync, enc->gpsimd, exp(bias)->
  gpsimd, out->sync) to parallelize descriptor generation.
- Normalization recip/broadcast/mul sits off the group critical chain
  (consumed only by the deferred output projection).
"""

import sys

try:
    import concourse.bass as bass
except ImportError:
    sys.path.insert(0, "/opt/trn_rl_repo")
    import concourse.bass as bass

import numpy as np
import ml_dtypes
_bf16 = ml_dtypes.bfloat16

import concourse.mybir as mybir
from concourse import bacc
from concourse.tile import TileContext
from concourse.bass_utils import run_bass_kernel_spmd

F32 = mybir.dt.float32
F16 = mybir.dt.float16
BF16 = mybir.dt.bfloat16

# Problem sizes (hardcoded per spec)
B, NQ, NKV = 4, 2048, 2048
D_MODEL, N_HEADS, D_K = 1024, 16, 64
N_CORES = 8
HPC = N_HEADS // N_CORES          # heads per core = 2
DH = HPC * D_K                    # 128 partition rows of per-core head dims

QW = 512                          # flash q window
KT = 128                          # k tile (partition dim of S^T)
LAG = 3                           # ctx matmul software-pipeline depth


def build_kernel(b=B, nq=NQ, nkv=NKV, d_model=D_MODEL, reps=1):
    nc = bacc.Bacc("TRN2", target_bir_lowering=False, debug=False,
                   num_devices=N_CORES)

    n_m = d_model // 128          # model-dim tiles (8)
    n_qw = nq // QW               # flash q windows (4)
    n_kt = nkv // KT              # k tiles (16)

    xT = nc.dram_tensor("xT", [b, d_model, nq], F16, kind="ExternalInput")
    encT = nc.dram_tensor("encT", [b, d_model, nkv], F16, kind="ExternalInput")
    # exp(position_bias), pre-tiled: [qw, kt, p, h*QW+q]
    ebias = nc.dram_tensor("ebias", [n_qw, n_kt, KT, HPC * QW], BF16,
                           kind="ExternalInput")
    wq = nc.dram_tensor("wq", [d_model, DH], F16, kind="ExternalInput")
    wk = nc.dram_tensor("wk", [d_model, DH], F16, kind="ExternalInput")
    wv = nc.dram_tensor("wv", [d_model, DH], F16, kind="ExternalInput")
    wo = nc.dram_tensor("wo", [DH, d_model], F16, kind="ExternalInput")
    ident16 = nc.dram_tensor("ident16", [128, 128], F16, kind="ExternalInput")
    out = nc.dram_tensor("out", [b, nq, d_model], F16, kind="ExternalOutput")

    with TileContext(nc) as tc:
        with (
            tc.tile_pool(name="cst", bufs=1) as cst,
            tc.tile_pool(name="wpool", bufs=1) as wpool,
            tc.tile_pool(name="qkv", bufs=1) as qkv,
            tc.tile_pool(name="stage", bufs=10) as stage,
            tc.tile_pool(name="sbias", bufs=20) as sbias,
            tc.tile_pool(name="sattn", bufs=4 * LAG) as sattn,
            tc.tile_pool(name="sctx", bufs=2 * b) as sctx,
            tc.tile_pool(name="vtstage", bufs=2) as vtstage,
            tc.tile_pool(name="sout", bufs=3) as sout,
            tc.tile_pool(name="ssmall", bufs=6) as ssmall,
            tc.tile_pool(name="psbig", bufs=3, space="PSUM") as psbig,
            tc.tile_pool(name="ps_u", bufs=1, space="PSUM") as ps_u,
        ):
            # ---- constants & weights ----
            ident = cst.tile([128, 128], F16, tag="ident")
            nc.sync.dma_start(out=ident, in_=ident16[:, :])

            wq_sb = wpool.tile([128, n_m * DH], F16, tag="wq")
            wk_sb = wpool.tile([128, n_m * DH], F16, tag="wk")
            wv_sb = wpool.tile([128, n_m * DH], F16, tag="wv")
            for m in range(n_m):
                nc.sync.dma_start(out=wq_sb[:, m * DH:(m + 1) * DH],
                                  in_=wq[m * 128:(m + 1) * 128, :])
                nc.sync.dma_start(out=wk_sb[:, m * DH:(m + 1) * DH],
                                  in_=wk[m * 128:(m + 1) * 128, :])
                nc.sync.dma_start(out=wv_sb[:, m * DH:(m + 1) * DH],
                                  in_=wv[m * 128:(m + 1) * 128, :])
            wo_sb = wpool.tile([128, d_model], F16, tag="wo")
            nc.sync.dma_start(out=wo_sb, in_=wo[:, :])

            qT_sb = qkv.tile([128, b * nq], F16, tag="qT")
            kT_sb = qkv.tile([128, b * nkv], F16, tag="kT")
            # pair-packed Vones tiles: [h0 V(64) | ones | h1 V(64) | ones]
            vones = {}
            for bi in range(b):
                for kt in range(n_kt):
                    vones[(bi, kt)] = qkv.tile(
                        [128, 2 * (D_K + 1)], BF16, tag=f"v_{bi}_{kt}",
                        name=f"v_{bi}_{kt}")

            lp = nc.allow_low_precision(reason="fp16/bf16 attention pipeline")
            lp.__enter__()
            for rep in range(reps):
                emit_body(nc, tc, rep, b, nq, nkv, d_model, n_m, n_qw, n_kt,
                          stage, sbias, sattn, sctx, vtstage, sout, ssmall,
                          psbig, ps_u, qT_sb, kT_sb, vones, wq_sb, wk_sb,
                          wv_sb, wo_sb, ident, xT, encT, ebias, out)
            lp.__exit__(None, None, None)
    nc.compile()
    return nc


def emit_body(nc, tc, rep, b, nq, nkv, d_model, n_m, n_qw, n_kt,
              stage, sbias, sattn, sctx, vtstage, sout, ssmall,
              psbig, ps_u, qT_sb, kT_sb, vones, wq_sb, wk_sb,
              wv_sb, wo_sb, ident, xT, encT, ebias, out):
            import concourse.mybir as mybir

            # ---- projections for one batch (emitted lazily) ----
            def emit_proj(bi):
                # Q^T: [128,1024] input slabs per (pw, m); pool pipelines
                for pw in range(nq // 1024):
                    q_ps = psbig.tile([128, 1024], F32, tag="big",
                                      name=f"qps_{rep}_{bi}_{pw}")
                    for m in range(n_m):
                        xt = stage.tile([128, 1024], F16, tag="stage",
                                        name=f"x_{rep}_{bi}_{pw}_{m}")
                        nc.sync.dma_start(
                            out=xt,
                            in_=xT[bi, m * 128:(m + 1) * 128,
                                   pw * 1024:(pw + 1) * 1024])
                        for s in range(2):
                            nc.tensor.matmul(
                                q_ps[:, s * 512:(s + 1) * 512],
                                wq_sb[:, m * DH:(m + 1) * DH],
                                xt[:, s * 512:(s + 1) * 512],
                                start=(m == 0), stop=(m == n_m - 1))
                    nc.scalar.copy(
                        qT_sb[:, bi * nq + pw * 1024:
                              bi * nq + (pw + 1) * 1024],
                        q_ps)
                # K^T and V^T
                for pw in range(nkv // 1024):
                    k_ps = psbig.tile([128, 1024], F32, tag="big",
                                      name=f"kps_{rep}_{bi}_{pw}")
                    v_ps = psbig.tile([128, 1024], F32, tag="big",
                                      name=f"vps_{rep}_{bi}_{pw}")
                    for m in range(n_m):
                        et = stage.tile([128, 1024], F16, tag="stage",
                                        name=f"e_{rep}_{bi}_{pw}_{m}")
                        nc.gpsimd.dma_start(
                            out=et,
                            in_=encT[bi, m * 128:(m + 1) * 128,
                                     pw * 1024:(pw + 1) * 1024])
                        for s in range(2):
                            nc.tensor.matmul(
                                k_ps[:, s * 512:(s + 1) * 512],
                                wk_sb[:, m * DH:(m + 1) * DH],
                                et[:, s * 512:(s + 1) * 512],
                                start=(m == 0), stop=(m == n_m - 1))
                            nc.tensor.matmul(
                                v_ps[:, s * 512:(s + 1) * 512],
                                wv_sb[:, m * DH:(m + 1) * DH],
                                et[:, s * 512:(s + 1) * 512],
                                start=(m == 0), stop=(m == n_m - 1))
                    nc.scalar.copy(
                        kT_sb[:, bi * nkv + pw * 1024:
                              bi * nkv + (pw + 1) * 1024],
                        k_ps)
                    vt_win = vtstage.tile([128, 1024], F16, tag="vtw")
                    nc.scalar.copy(vt_win, v_ps)
                    # V^T -> V tiles via PE transpose into one F16 PSUM
                    # tile (slots sized for [128,1024] F32, so F16 fits)
                    vtbig = psbig.tile([128, 1024], F16, tag="big",
                                       name=f"vtb_{rep}_{bi}_{pw}")
                    for s in range(1024 // KT):
                        kt = pw * (1024 // KT) + s
                        nc.tensor.transpose(
                            vtbig[:, s * KT:(s + 1) * KT],
                            vt_win[:, s * KT:(s + 1) * KT], ident)
                        vt = vones[(bi, kt)]
                        for h in range(HPC):
                            o = h * (D_K + 1)
                            nc.vector.tensor_copy(
                                vt[:, o:o + D_K],
                                vtbig[:, s * KT + h * D_K:
                                      s * KT + (h + 1) * D_K])
                            nc.vector.memset(
                                vt[:, o + D_K:o + D_K + 1], 1.0)

            # ---- output projection for one q-window (deferred) ----
            def emit_wo(pend):
                pq0, pctx = pend
                for bi in range(b):
                    for qs in range(QW // 128):
                        o_ps = psbig.tile([128, d_model], F32, tag="big",
                                          name=f"ops_{rep}_{pq0}_{bi}_{qs}")
                        for e in range(d_model // 512):
                            nc.tensor.matmul(
                                o_ps[:, e * 512:(e + 1) * 512],
                                pctx[bi][:, qs * 128:(qs + 1) * 128],
                                wo_sb[:, e * 512:(e + 1) * 512],
                                start=True, stop=True)
                        o_sb = sout.tile([128, d_model], F16, tag="out")
                        if (bi + qs) % 2 == 0:
                            nc.vector.tensor_copy(o_sb, o_ps)
                        else:
                            nc.scalar.copy(o_sb, o_ps)
                        nc.sync.dma_start(
                            out=out[bi, pq0 + qs * 128:
                                    pq0 + (qs + 1) * 128, :],
                            in_=o_sb)

            # ---- phase B: flash attention, software-pipelined ----
            if True:
                pending_wo = None
                for qw in range(n_qw):
                    q0 = qw * QW
                    # preload exp(bias) for the whole q-window
                    eb_sb = {}
                    for kt in range(n_kt):
                        eb_sb[kt] = sbias.tile(
                            [128, HPC * QW], BF16, tag="bias",
                            name=f"eb_{rep}_{qw}_{kt}")
                        nc.scalar.dma_start(
                            out=eb_sb[kt], in_=ebias[qw, kt])
                    ctx_t = [sctx.tile([128, QW], F16, tag="ctx",
                                       name=f"ctx_{rep}_{qw}_{bi}")
                             for bi in range(b)]
                    for bi in range(b):
                        if qw == 0:
                            emit_proj(bi)
                        u = ps_u.tile([D_K + 1, 2 * QW], F32, tag="u",
                                      name=f"u_{rep}_{qw}_{bi}")
                        # pre-zero + start=False accumulating matmuls
                        nc.vector.memset(u, 0.0)
                        pend = []

                        def issue_ctx(item):
                            kt_i, attnb_i = item
                            for h in range(HPC):
                                o = h * (D_K + 1)
                                nc.tensor.matmul(
                                    u[:, h * QW:(h + 1) * QW],
                                    vones[(bi, kt_i)][:, o:o + D_K + 1],
                                    attnb_i[:, h * QW:(h + 1) * QW],
                                    start=False, stop=(kt_i == n_kt - 1),
                                    skip_group_check=True)

                        for kt in range(n_kt):
                            if (bi == b - 1 and kt == n_kt - 1
                                    and pending_wo is not None):
                                emit_wo(pending_wo)
                                pending_wo = None
                            s_g = psbig.tile([128, 2 * QW], F32, tag="big",
                                             name="sg")
                            # two heads' score matmuls adjacent: disjoint PE
                            # row groups, disjoint PSUM banks
                            for h in range(HPC):
                                hp = h * D_K
                                nc.tensor.matmul(
                                    s_g[:, h * QW:(h + 1) * QW],
                                    kT_sb[hp:hp + D_K,
                                          bi * nkv + kt * KT:
                                          bi * nkv + (kt + 1) * KT],
                                    qT_sb[hp:hp + D_K,
                                          bi * nq + q0:bi * nq + q0 + QW],
                                    start=True, stop=True)
                            attn = sattn.tile([128, 2 * QW], BF16,
                                              tag="attn", name="at")
                            nc.scalar.activation(
                                attn, s_g, mybir.ActivationFunctionType.Exp)
                            attnb = sattn.tile([128, 2 * QW], BF16,
                                               tag="attn", name="ab")
                            nc.vector.tensor_mul(attnb, attn, eb_sb[kt])
                            pend.append((kt, attnb))
                            if len(pend) > LAG:
                                issue_ctx(pend.pop(0))
                        for item in pend:
                            issue_ctx(item)
                        # normalization (off the group critical chain)
                        for h in range(HPC):
                            hp = h * D_K
                            usrc = u[:, h * QW:(h + 1) * QW]
                            recip = ssmall.tile([1, QW], F32, tag="recip",
                                                name=f"recip_{rep}_{h}")
                            nc.vector.reciprocal(recip,
                                                 usrc[D_K:D_K + 1, :])
                            rb = ssmall.tile([D_K, QW], F32, tag="rb",
                                             name=f"rb_{rep}_{h}")
                            nc.gpsimd.partition_broadcast(rb, recip)
                            nc.vector.tensor_mul(
                                ctx_t[bi][hp:hp + D_K, :],
                                usrc[0:D_K, :], rb)
                    pending_wo = (q0, ctx_t)
                emit_wo(pending_wo)


_NC_CACHE = {}


def _get_nc():
    if "nc" not in _NC_CACHE:
        _NC_CACHE["nc"] = build_kernel()
    return _NC_CACHE["nc"]


def _prep_inputs(x, encoding, position_bias, Wq, Wk, Wv, Wo):
    x = np.asarray(x, np.float32)
    encoding = np.asarray(encoding, np.float32)
    position_bias = np.asarray(position_bias, np.float32)

    xT = np.ascontiguousarray(
        x.transpose(0, 2, 1)).astype(np.float16)
    encT = np.ascontiguousarray(
        encoding.transpose(0, 2, 1)).astype(np.float16)
    ident16 = np.eye(128, dtype=np.float16)

    n_qw = NQ // QW
    n_kt = NKV // KT

    in_maps = []
    for c in range(N_CORES):
        h0 = c * HPC
        # exp(bias) pre-tiled: [qw, kt, p, h*QW+q] =
        #   exp(bias^T[h, kt*KT + p, qw*QW + q]); position_bias[0,h] is [q,k]
        eb = np.exp(position_bias[0, h0:h0 + HPC])           # [h, q, k]
        eb = eb.reshape(HPC, n_qw, QW, n_kt, KT)             # h,qw,q,kt,p
        eb = np.ascontiguousarray(eb.transpose(1, 3, 4, 0, 2)).reshape(
            n_qw, n_kt, KT, HPC * QW).astype(_bf16)
        in_maps.append({
            "xT": xT,
            "encT": encT,
            "ebias": eb,
            "wq": np.ascontiguousarray(
                Wq[:, h0 * D_K:(h0 + HPC) * D_K]).astype(np.float16),
            "wk": np.ascontiguousarray(
                Wk[:, h0 * D_K:(h0 + HPC) * D_K]).astype(np.float16),
            "wv": np.ascontiguousarray(
                Wv[:, h0 * D_K:(h0 + HPC) * D_K]).astype(np.float16),
            "wo": np.ascontiguousarray(
                Wo[h0 * D_K:(h0 + HPC) * D_K, :]).astype(np.float16),
            "ident16": ident16,
        })
    return in_maps


def kernel(x, encoding, position_bias, Wq, Wk, Wv, Wo):
    in_maps = _prep_inputs(x, encoding, position_bias,
                           np.asarray(Wq, np.float32),
                           np.asarray(Wk, np.float32),
                           np.asarray(Wv, np.float32),
                           np.asarray(Wo, np.float32))
    nc = _get_nc()
    res = run_bass_kernel_spmd(nc, in_maps, list(range(N_CORES)))
    acc = res.results[0]["out"].astype(np.float32)
    for c in range(1, N_CORES):
        acc = acc + res.results[c]["out"].astype(np.float32)
    return acc
